# revision 28
# baseline (speedup 1.0000x reference)
"""Trainium2 Bass kernel for BasicAttention.

reference math (fp32):
  xf = x.reshape(b, din, hw)               # b=4, din=256, hw=4096
  Q = q_w @ xf   [b, 64, hw]
  K = k_w @ xf   [b, 64, hw]
  V = v_w @ xf   [b, 256, hw]
  S = Q^T K      [b, hw, hw]
  A = softmax(S, axis=-1)
  z = (A @ V^T)^T -> [b, 256, h, w]

Sharding: 8 cores = (batch b in 0..4) x (query half in 0..2). Each core gets
its batch's full xf with columns rotated so its 2048 queries come first
(attention is permutation-invariant over keys, so K/V built from the rotated
xf give identical outputs).

Dtypes: x / weights / Q / K in fp16, S psum fp32, exp -> bf16 (|S| < ~45 so
exp(S) needs bf16's e8 exponent; no max-subtraction pass), V tiles bf16,
Z matmuls bf16 x bf16 -> fp32 psum, output written fp16 (host casts to f32).
End-to-end rel err ~6e-3 vs the 2e-2 gate.

Per-core dataflow:
  - For each 512-query ptile: per 128-key chunk: S^T psum tile [keys=128,
    queries=512] = K_chunk(lhsT) @ Q; exp on ACT straight out of PSUM into
    bf16; two Z matmuls accumulate V^T_chunk^T @ expS into psum [dv=128,512].
  - Projections are emitted just-in-time inside ptile 0's key loop so the
    PE queue never sits behind a long prelude; Q tiles for ptile p are
    emitted at the top of ptile p.
  - Softmax denominators: bf16 pair/quad tree on DVE, fp32 accumulator;
    a ones[128,128] matmul replicates the key-sum across all psum
    partitions; ACT evicts it to SBUF; two DVE divide ops produce the
    normalized fp16 output tiles directly (no reciprocal / broadcast).
  - Each ptile tail is deferred into the next ptile's stream; the last
    ptile's denominator chain is hoisted before its final Z matmuls so only
    the two divides + output DMA sit after the last matmul.
"""

import sys
import os

sys.path.insert(0, "/opt/trn_rl_repo")

import numpy as np

B, DIN, H, W = 4, 256, 64, 64
HW = H * W            # 4096 keys
DK, DV = 64, 256
PQ = HW // 2          # 2048 queries per core
PT = 512              # query tile (psum free dim)
QC = 128              # key chunk (contraction tile)
NPT = PQ // PT        # 4
NQC = HW // QC        # 32
N_CORES = 8

_cache = {}


def _build():
    if "nc" in _cache:
        return _cache["nc"]

    from contextlib import ExitStack
    import concourse.tile as tile
    from concourse import bacc, mybir

    f32 = mybir.dt.float32
    f32r = mybir.dt.float32r
    f16 = mybir.dt.float16
    bf16 = mybir.dt.bfloat16

    nc = bacc.Bacc("TRN2", target_bir_lowering=False, debug=False,
                   num_devices=N_CORES)

    xb = nc.dram_tensor("xb", [DIN, HW], f16, kind="ExternalInput").ap()
    qwT = nc.dram_tensor("qwT", [DIN, DK], f16, kind="ExternalInput").ap()
    kwT = nc.dram_tensor("kwT", [DIN, DK], f16, kind="ExternalInput").ap()
    vwT = nc.dram_tensor("vwT", [DIN, DV], f16, kind="ExternalInput").ap()
    zout = nc.dram_tensor("zout", [DV, PQ], f16, kind="ExternalOutput").ap()

    with tile.TileContext(nc) as tc, ExitStack() as ctx:
        singles = ctx.enter_context(tc.tile_pool(name="singles", bufs=1))
        vt_pool = ctx.enter_context(tc.tile_pool(name="vt_pool", bufs=NQC))
        exps_pool = ctx.enter_context(tc.tile_pool(name="exps_pool", bufs=8))
        sum_pool = ctx.enter_context(tc.tile_pool(name="sum_pool", bufs=2))
        out_pool = ctx.enter_context(tc.tile_pool(name="out_pool", bufs=4))
        ps_s = ctx.enter_context(tc.tile_pool(name="ps_s", bufs=4, space="PSUM"))
        ps_z = ctx.enter_context(tc.tile_pool(name="ps_z", bufs=4, space="PSUM"))

        # ---- weights via SWDGE (parallel with the big x loads below) ----
        w_q0 = singles.tile([128, DK], f16)
        w_q1 = singles.tile([128, DK], f16)
        w_k0 = singles.tile([128, DK], f16)
        w_k1 = singles.tile([128, DK], f16)
        w_v0 = singles.tile([128, DV], f16)
        w_v1 = singles.tile([128, DV], f16)
        nc.sync.dma_start(out=w_k0, in_=kwT[0:128, :])
        nc.sync.dma_start(out=w_k1, in_=kwT[128:256, :])
        nc.scalar.dma_start(out=w_q0, in_=qwT[0:128, :])
        nc.scalar.dma_start(out=w_q1, in_=qwT[128:256, :])
        nc.scalar.dma_start(out=w_v0, in_=vwT[0:128, :])
        nc.scalar.dma_start(out=w_v1, in_=vwT[128:256, :])

        # ones weights [128, 128] for the key-sum matmul: the column sums land
        # replicated on every psum partition, so the reciprocal runs on all
        # 128 DVE lanes and feeds the normalize muls directly (no broadcast).
        ones_f = singles.tile([128, 128], f32)
        nc.vector.memset(ones_f, 1.0)
        ones_c = singles.tile([128, 128], f32r)
        nc.scalar.copy(ones_c, ones_f)

        q_sb = singles.tile([DK, PQ], f16)
        k_sb = singles.tile([DK, HW], f16)
        xf0 = singles.tile([128, HW], f16)
        xf1 = singles.tile([128, HW], f16)

        # ---- chunked x load spread over four DGE queues ----
        CHW = 512                        # columns per chunk
        NCH = HW // CHW                  # 8 chunks
        dma_engs = [nc.sync, nc.scalar, nc.gpsimd]
        for g in range(NCH):
            sl = slice(g * CHW, (g + 1) * CHW)
            eng = dma_engs[g % 3]
            eng.dma_start(out=xf0[:, sl], in_=xb[0:128, sl])
            eng.dma_start(out=xf1[:, sl], in_=xb[128:256, sl])

        vt = [None] * NQC

        def proj_k(c):
            sl = slice(c * CHW, (c + 1) * CHW)
            pk = ps_s.tile([DK, CHW], f32, name=f"ps_k{c}", tag="ps_s")
            nc.tensor.matmul(pk, w_k0, xf0[:, sl], start=True, stop=False)
            nc.tensor.matmul(pk, w_k1, xf1[:, sl], start=False, stop=True)
            nc.vector.tensor_copy(k_sb[:, sl], pk)

        def proj_v(c):
            for qc in range(c * CHW // QC, (c + 1) * CHW // QC):
                pv = ps_s.tile([QC, DV], f32, name=f"ps_v{qc}", tag="ps_s")
                nc.tensor.matmul(pv, xf0[:, qc * QC:(qc + 1) * QC], w_v0,
                                 start=True, stop=False)
                nc.tensor.matmul(pv, xf1[:, qc * QC:(qc + 1) * QC], w_v1,
                                 start=False, stop=True)
                vt_t = vt_pool.tile([QC, DV], bf16, name=f"vt{qc}", tag="vt")
                if qc % 2 == 0:
                    nc.scalar.copy(vt_t, pv)
                else:
                    nc.vector.tensor_copy(vt_t, pv)
                vt[qc] = vt_t

        def proj_q(i):
            sl = slice(i * PT, (i + 1) * PT)
            pq = ps_s.tile([DK, PT], f32, name=f"ps_q{i}", tag="ps_s")
            nc.tensor.matmul(pq, w_q0, xf0[:, sl], start=True, stop=False)
            nc.tensor.matmul(pq, w_q1, xf1[:, sl], start=False, stop=True)
            nc.vector.tensor_copy(q_sb[:, sl], pq)

        proj_k(0)
        proj_q(0)

        # ---- attention main loop ----
        deferred = None
        for pt in range(NPT):
            last_pt = pt == NPT - 1
            qs = q_sb[:, pt * PT:(pt + 1) * PT]
            pz0 = ps_z.tile([128, PT], f32, name=f"pz0_{pt}", tag="pz")
            pz1 = ps_z.tile([128, PT], f32, name=f"pz1_{pt}", tag="pz")
            acc = sum_pool.tile([QC, PT], f32, name=f"acc_{pt}", tag="acc")

            def s_mm(qc, qs=qs, pt=pt):
                ps = ps_s.tile([QC, PT], f32, name=f"ps_{pt}_{qc}", tag="ps_s")
                nc.tensor.matmul(ps, k_sb[:, qc * QC:(qc + 1) * QC], qs,
                                 start=True, stop=True)
                return ps

            pend = [s_mm(i) for i in range(4)]

            def mk_exp(qc, pt=pt):
                e = exps_pool.tile([QC, PT], bf16,
                                   name=f"exps_{pt}_{qc}", tag="exps",
                                   bufs=12)
                nc.scalar.activation(e, pend.pop(0),
                                     func=mybir.ActivationFunctionType.Exp)
                return e

            E = {}
            if pt == 0:
                proj_v(0)
            E[0], E[1] = mk_exp(0), mk_exp(1)
            if pt == 0:
                proj_k(1)
                proj_v(1)
            pend.extend([s_mm(4), s_mm(5)])
            E[2], E[3] = mk_exp(2), mk_exp(3)
            if pt == 0:
                # extra upfront projection work: fills the PE pipe while the
                # first exps wait on the ACT table load at kernel start
                proj_k(2)
                proj_v(2)

            # denominator chain for this ptile: cast, ones-matmul (key sums
            # replicated on all 128 psum partitions), exact reciprocal on
            # 128 DVE lanes straight out of PSUM.
            def den_chain(pt=pt, acc=acc):
                accr = sum_pool.tile([QC, PT], f32r,
                                     name=f"accr{pt}", tag="accr")
                nc.scalar.copy(accr, acc)
                ps_den = ps_s.tile([128, PT], f32,
                                   name=f"ps_den{pt}", tag="ps_s")
                nc.tensor.matmul(ps_den, ones_c, accr, start=True, stop=True)
                den = sum_pool.tile([128, PT], f32, name=f"den{pt}", tag="den")
                nc.vector.reciprocal(den, ps_den)
                return den

            pairs = {}
            den = None

            def sum_tree(g, acc=acc, pairs=pairs, pt=pt, dve=False):
                p_t = exps_pool.tile([QC, PT], bf16,
                                     name=f"pair_{pt}_{g}", tag="pair")
                nc.vector.tensor_add(p_t, E[g], E[g + 1])
                pairs[g] = p_t
                if g % 4 == 2:
                    qd = exps_pool.tile([QC, PT], bf16,
                                        name=f"quad_{pt}_{g}", tag="quad")
                    eng = nc.vector if dve else nc.gpsimd
                    eng.tensor_add(qd, pairs[g - 2], pairs[g])
                    if g == 2:
                        nc.vector.tensor_copy(acc, qd)
                    else:
                        nc.vector.tensor_add(acc, acc, qd)

            if last_pt:
                # run the sum tree as soon as each E pair exists so the
                # denominator chain completes under the final Z matmuls
                sum_tree(0, dve=True)
                sum_tree(2, dve=True)

            for g in range(0, NQC, 2):
                if pt == 0 and g + 6 < NQC and (g + 6) % (CHW // QC) == 0:
                    c = (g + 6) * QC // CHW + 1
                    if c < NCH:
                        proj_k(c)
                        proj_v(c)
                if pt < NPT - 1 and g == 20:
                    # prefetch next ptile's Q projection so its eviction is
                    # done well before the boundary
                    proj_q(pt + 1)
                for h in range(2):
                    if g + 4 + h < NQC:
                        E[g + 4 + h] = mk_exp(g + 4 + h)
                if last_pt and g + 4 < NQC:
                    sum_tree(g + 4, dve=True)
                if last_pt and g == NQC - 6:
                    den = den_chain()
                for v, pz in ((0, pz0), (1, pz1)):
                    hs = (1, 0) if v == 0 else (0, 1)
                    for idx, h in enumerate(hs):
                        nc.tensor.matmul(pz,
                                         vt[g + h][:, v * 128:(v + 1) * 128],
                                         E[g + h],
                                         start=(g == 0 and idx == 0),
                                         stop=(g == NQC - 2 and idx == 1))
                for h in range(2):
                    if g + 6 + h < NQC:
                        pend.append(s_mm(g + 6 + h))
                if not last_pt:
                    sum_tree(g)
                if g == 4 and deferred is not None:
                    deferred()
                    deferred = None

            def make_tail(pt=pt, pz0=pz0, pz1=pz1, den=den, last=last_pt,
                          den_chain=den_chain, acc=acc):
                def tail():
                    d = den if last else den_chain(pt=pt, acc=acc)
                    out0 = out_pool.tile([128, PT], f16,
                                         name=f"out0_{pt}", tag="out")
                    out1 = out_pool.tile([128, PT], f16,
                                         name=f"out1_{pt}", tag="out")
                    nc.vector.tensor_mul(out0, pz0, d)
                    nc.vector.tensor_mul(out1, pz1, d)
                    nc.sync.dma_start(out=zout[0:128, pt * PT:(pt + 1) * PT],
                                      in_=out0)
                    nc.sync.dma_start(
                        out=zout[128:256, pt * PT:(pt + 1) * PT], in_=out1)
                return tail

            deferred = make_tail()
        deferred()

    nc.compile()
    _cache["nc"] = nc
    return nc


def _in_maps(x, q_w, k_w, v_w):
    xf = np.asarray(x, np.float32).reshape(B, DIN, HW)
    qwT = np.ascontiguousarray(np.asarray(q_w, np.float32).T.astype(np.float16))
    kwT = np.ascontiguousarray(np.asarray(k_w, np.float32).T.astype(np.float16))
    vwT = np.ascontiguousarray(np.asarray(v_w, np.float32).T.astype(np.float16))
    maps = []
    for c in range(N_CORES):
        b, half = divmod(c, 2)
        xbc = xf[b] if half == 0 else np.roll(xf[b], -PQ, axis=1)
        xbc = np.ascontiguousarray(xbc.astype(np.float16))
        maps.append({"xb": xbc, "qwT": qwT, "kwT": kwT, "vwT": vwT})
    return maps


def _gather(results):
    z = np.empty((B, DV, HW), np.float32)
    for c in range(N_CORES):
        b, half = divmod(c, 2)
        z[b][:, half * PQ:(half + 1) * PQ] = results[c]["zout"]
    return z.reshape(B, DV, H, W)


def _run(x, q_w, k_w, v_w, trace=False):
    from concourse import bass_utils
    nc = _build()
    res = bass_utils.run_bass_kernel_spmd(
        nc, _in_maps(x, q_w, k_w, v_w), core_ids=list(range(N_CORES)),
        trace=trace)
    return _gather(res.results), res


def kernel(x, q_w, k_w, v_w):
    z, _ = _run(x, q_w, k_w, v_w)
    return z


# revision 30
# speedup vs baseline: 1.0975x; 1.0975x over previous
"""Trainium2 Bass kernel for BasicAttention.

reference math (fp32):
  xf = x.reshape(b, din, hw)               # b=4, din=256, hw=4096
  Q = q_w @ xf   [b, 64, hw]
  K = k_w @ xf   [b, 64, hw]
  V = v_w @ xf   [b, 256, hw]
  S = Q^T K      [b, hw, hw]
  A = softmax(S, axis=-1)
  z = (A @ V^T)^T -> [b, 256, h, w]

Sharding: 8 cores = (batch b in 0..4) x (query half in 0..2). Each core gets
its batch's full xf with columns rotated so its 2048 queries come first
(attention is permutation-invariant over keys, so K/V built from the rotated
xf give identical outputs).

Dtypes: x / weights / Q / K in fp16, S psum fp32, exp -> bf16 (|S| < ~45 so
exp(S) needs bf16's e8 exponent; no max-subtraction pass), V tiles bf16,
Z matmuls bf16 x bf16 -> fp32 psum, output written fp16 (host casts to f32).
End-to-end rel err ~6e-3 vs the 2e-2 gate.

Per-core dataflow:
  - For each 512-query ptile: per 128-key chunk: S^T psum tile [keys=128,
    queries=512] = K_chunk(lhsT) @ Q; exp on ACT straight out of PSUM into
    bf16; two Z matmuls accumulate V^T_chunk^T @ expS into psum [dv=128,512].
  - Projections are emitted just-in-time inside ptile 0's key loop so the
    PE queue never sits behind a long prelude; Q tiles for ptile p are
    emitted at the top of ptile p.
  - Softmax denominators: bf16 pair/quad tree on DVE, fp32 accumulator;
    a ones[128,128] matmul replicates the key-sum across all psum
    partitions; ACT evicts it to SBUF; two DVE divide ops produce the
    normalized fp16 output tiles directly (no reciprocal / broadcast).
  - Each ptile tail is deferred into the next ptile's stream; the last
    ptile's denominator chain is hoisted before its final Z matmuls so only
    the two divides + output DMA sit after the last matmul.
"""

import sys
import os

sys.path.insert(0, "/opt/trn_rl_repo")

import numpy as np

B, DIN, H, W = 4, 256, 64, 64
HW = H * W            # 4096 keys
DK, DV = 64, 256
PQ = HW // 2          # 2048 queries per core
PT = 512              # query tile (psum free dim)
QC = 128              # key chunk (contraction tile)
NPT = PQ // PT        # 4
NQC = HW // QC        # 32
N_CORES = 8

_cache = {}


def _build():
    if "nc" in _cache:
        return _cache["nc"]

    from contextlib import ExitStack
    import concourse.tile as tile
    from concourse import bacc, mybir

    f32 = mybir.dt.float32
    f32r = mybir.dt.float32r
    f16 = mybir.dt.float16
    bf16 = mybir.dt.bfloat16

    nc = bacc.Bacc("TRN2", target_bir_lowering=False, debug=False,
                   num_devices=N_CORES)

    xb = nc.dram_tensor("xb", [DIN, HW], f16, kind="ExternalInput").ap()
    qwT = nc.dram_tensor("qwT", [DIN, DK], f16, kind="ExternalInput").ap()
    kwT = nc.dram_tensor("kwT", [DIN, DK], f16, kind="ExternalInput").ap()
    vwT = nc.dram_tensor("vwT", [DIN, DV], f16, kind="ExternalInput").ap()
    zout = nc.dram_tensor("zout", [DV, PQ], f16, kind="ExternalOutput").ap()

    with tile.TileContext(nc) as tc, ExitStack() as ctx:
        singles = ctx.enter_context(tc.tile_pool(name="singles", bufs=1))
        vt_pool = ctx.enter_context(tc.tile_pool(name="vt_pool", bufs=NQC))
        exps_pool = ctx.enter_context(tc.tile_pool(name="exps_pool", bufs=8))
        sum_pool = ctx.enter_context(tc.tile_pool(name="sum_pool", bufs=2))
        out_pool = ctx.enter_context(tc.tile_pool(name="out_pool", bufs=4))
        ps_s = ctx.enter_context(tc.tile_pool(name="ps_s", bufs=4, space="PSUM"))
        ps_z = ctx.enter_context(tc.tile_pool(name="ps_z", bufs=4, space="PSUM"))

        # ---- weights via SWDGE (parallel with the big x loads below) ----
        w_q0 = singles.tile([128, DK], f16)
        w_q1 = singles.tile([128, DK], f16)
        w_k0 = singles.tile([128, DK], f16)
        w_k1 = singles.tile([128, DK], f16)
        w_v0 = singles.tile([128, DV], f16)
        w_v1 = singles.tile([128, DV], f16)
        nc.sync.dma_start(out=w_k0, in_=kwT[0:128, :])
        nc.sync.dma_start(out=w_k1, in_=kwT[128:256, :])
        nc.scalar.dma_start(out=w_q0, in_=qwT[0:128, :])
        nc.scalar.dma_start(out=w_q1, in_=qwT[128:256, :])
        nc.scalar.dma_start(out=w_v0, in_=vwT[0:128, :])
        nc.scalar.dma_start(out=w_v1, in_=vwT[128:256, :])

        # ones weights [128, 128] for the key-sum matmul: the column sums land
        # replicated on every psum partition, so the reciprocal runs on all
        # 128 DVE lanes and feeds the normalize muls directly (no broadcast).
        ones_f = singles.tile([128, 128], f32)
        nc.vector.memset(ones_f, 1.0)
        ones_c = singles.tile([128, 128], f32r)
        nc.scalar.copy(ones_c, ones_f)

        q_sb = singles.tile([DK, PQ], f16)
        k_sb = singles.tile([DK, HW], f16)
        xf0 = singles.tile([128, HW], f16)
        xf1 = singles.tile([128, HW], f16)

        # ---- chunked x load spread over four DGE queues ----
        CHW = 512                        # columns per chunk
        NCH = HW // CHW                  # 8 chunks
        dma_engs = [nc.sync, nc.scalar, nc.gpsimd]
        for g in range(NCH):
            sl = slice(g * CHW, (g + 1) * CHW)
            eng = dma_engs[g % 3]
            eng.dma_start(out=xf0[:, sl], in_=xb[0:128, sl])
            eng.dma_start(out=xf1[:, sl], in_=xb[128:256, sl])

        vt = [None] * NQC

        def proj_k(c):
            sl = slice(c * CHW, (c + 1) * CHW)
            pk = ps_s.tile([DK, CHW], f32, name=f"ps_k{c}", tag="ps_s")
            nc.tensor.matmul(pk, w_k0, xf0[:, sl], start=True, stop=False)
            nc.tensor.matmul(pk, w_k1, xf1[:, sl], start=False, stop=True)
            nc.vector.tensor_copy(k_sb[:, sl], pk)

        def proj_v(c):
            for qc in range(c * CHW // QC, (c + 1) * CHW // QC):
                pv = ps_s.tile([QC, DV], f32, name=f"ps_v{qc}", tag="ps_s")
                nc.tensor.matmul(pv, xf0[:, qc * QC:(qc + 1) * QC], w_v0,
                                 start=True, stop=False)
                nc.tensor.matmul(pv, xf1[:, qc * QC:(qc + 1) * QC], w_v1,
                                 start=False, stop=True)
                vt_t = vt_pool.tile([QC, DV], bf16, name=f"vt{qc}", tag="vt")
                nc.vector.tensor_copy(vt_t, pv)
                vt[qc] = vt_t

        def proj_q(i):
            sl = slice(i * PT, (i + 1) * PT)
            pq = ps_s.tile([DK, PT], f32, name=f"ps_q{i}", tag="ps_s")
            nc.tensor.matmul(pq, w_q0, xf0[:, sl], start=True, stop=False)
            nc.tensor.matmul(pq, w_q1, xf1[:, sl], start=False, stop=True)
            nc.vector.tensor_copy(q_sb[:, sl], pq)

        proj_k(0)
        proj_q(0)

        # ---- attention main loop ----
        deferred = None
        for pt in range(NPT):
            last_pt = pt == NPT - 1
            qs = q_sb[:, pt * PT:(pt + 1) * PT]
            pz0 = ps_z.tile([128, PT], f32, name=f"pz0_{pt}", tag="pz")
            pz1 = ps_z.tile([128, PT], f32, name=f"pz1_{pt}", tag="pz")
            acc = sum_pool.tile([QC, PT], f32, name=f"acc_{pt}", tag="acc")

            def s_mm(qc, qs=qs, pt=pt):
                ps = ps_s.tile([QC, PT], f32, name=f"ps_{pt}_{qc}", tag="ps_s")
                nc.tensor.matmul(ps, k_sb[:, qc * QC:(qc + 1) * QC], qs,
                                 start=True, stop=True)
                return ps

            pend = [s_mm(i) for i in range(4)]

            def mk_exp(qc, pt=pt):
                e = exps_pool.tile([QC, PT], bf16,
                                   name=f"exps_{pt}_{qc}", tag="exps",
                                   bufs=12)
                nc.scalar.activation(e, pend.pop(0),
                                     func=mybir.ActivationFunctionType.Exp)
                return e

            E = {}
            if pt == 0:
                proj_v(0)
            E[0], E[1] = mk_exp(0), mk_exp(1)
            if pt == 0:
                proj_k(1)
                proj_v(1)
            pend.extend([s_mm(4), s_mm(5)])
            E[2], E[3] = mk_exp(2), mk_exp(3)
            if pt == 0:
                # extra upfront projection work: fills the PE pipe while the
                # first exps wait on the ACT table load at kernel start
                proj_k(2)
                proj_v(2)

            # denominator chain for this ptile: cast, ones-matmul (key sums
            # replicated on all 128 psum partitions), exact reciprocal on
            # 128 DVE lanes straight out of PSUM.
            def den_chain(pt=pt, acc=acc):
                accr = sum_pool.tile([QC, PT], f32r,
                                     name=f"accr{pt}", tag="accr")
                nc.scalar.copy(accr, acc)
                ps_den = ps_s.tile([128, PT], f32,
                                   name=f"ps_den{pt}", tag="ps_s")
                nc.tensor.matmul(ps_den, ones_c, accr, start=True, stop=True)
                # 1/x as rsqrt(x)^2 on ACT (x > 0): ~2x faster than the DVE
                # reciprocal and keeps DVE free for the sum tree. Max rel err
                # ~6e-3 on the denominator (measured), fine vs the 2e-2 gate.
                rs = sum_pool.tile([128, PT], f32, name=f"rs{pt}", tag="rs")
                nc.scalar.activation(rs, ps_den,
                                     func=mybir.ActivationFunctionType.Abs_reciprocal_sqrt)
                den = sum_pool.tile([128, PT], f32, name=f"den{pt}", tag="den")
                nc.scalar.square(den, rs)
                return den

            pairs = {}
            den = None

            def sum_tree(g, acc=acc, pairs=pairs, pt=pt, dve=False):
                p_t = exps_pool.tile([QC, PT], bf16,
                                     name=f"pair_{pt}_{g}", tag="pair")
                nc.vector.tensor_add(p_t, E[g], E[g + 1])
                pairs[g] = p_t
                if g % 4 == 2:
                    qd = exps_pool.tile([QC, PT], bf16,
                                        name=f"quad_{pt}_{g}", tag="quad")
                    eng = nc.vector if dve else nc.gpsimd
                    eng.tensor_add(qd, pairs[g - 2], pairs[g])
                    if g == 2:
                        nc.vector.tensor_copy(acc, qd)
                    else:
                        nc.vector.tensor_add(acc, acc, qd)

            if last_pt:
                # run the sum tree as soon as each E pair exists so the
                # denominator chain completes under the final Z matmuls
                sum_tree(0, dve=True)
                sum_tree(2, dve=True)

            for g in range(0, NQC, 2):
                if pt == 0 and g + 6 < NQC and (g + 6) % (CHW // QC) == 0:
                    c = (g + 6) * QC // CHW + 1
                    if c < NCH:
                        proj_k(c)
                        proj_v(c)
                if pt < NPT - 1 and g == 20:
                    # prefetch next ptile's Q projection so its eviction is
                    # done well before the boundary
                    proj_q(pt + 1)
                for h in range(2):
                    if g + 4 + h < NQC:
                        E[g + 4 + h] = mk_exp(g + 4 + h)
                if last_pt and g + 4 < NQC:
                    sum_tree(g + 4, dve=True)
                if last_pt and g == NQC - 6:
                    den = den_chain()
                for v, pz in ((0, pz0), (1, pz1)):
                    hs = (1, 0) if v == 0 else (0, 1)
                    for idx, h in enumerate(hs):
                        nc.tensor.matmul(pz,
                                         vt[g + h][:, v * 128:(v + 1) * 128],
                                         E[g + h],
                                         start=(g == 0 and idx == 0),
                                         stop=(g == NQC - 2 and idx == 1))
                for h in range(2):
                    if g + 6 + h < NQC:
                        pend.append(s_mm(g + 6 + h))
                if not last_pt:
                    sum_tree(g)
                if g == 4 and deferred is not None:
                    deferred()
                    deferred = None

            def make_tail(pt=pt, pz0=pz0, pz1=pz1, den=den, last=last_pt,
                          den_chain=den_chain, acc=acc):
                def tail():
                    d = den if last else den_chain(pt=pt, acc=acc)
                    out0 = out_pool.tile([128, PT], f16,
                                         name=f"out0_{pt}", tag="out")
                    out1 = out_pool.tile([128, PT], f16,
                                         name=f"out1_{pt}", tag="out")
                    nc.vector.tensor_mul(out0, pz0, d)
                    nc.vector.tensor_mul(out1, pz1, d)
                    nc.sync.dma_start(out=zout[0:128, pt * PT:(pt + 1) * PT],
                                      in_=out0)
                    nc.sync.dma_start(
                        out=zout[128:256, pt * PT:(pt + 1) * PT], in_=out1)
                return tail

            deferred = make_tail()
        deferred()

    nc.compile()
    _cache["nc"] = nc
    return nc


def _in_maps(x, q_w, k_w, v_w):
    xf = np.asarray(x, np.float32).reshape(B, DIN, HW)
    qwT = np.ascontiguousarray(np.asarray(q_w, np.float32).T.astype(np.float16))
    kwT = np.ascontiguousarray(np.asarray(k_w, np.float32).T.astype(np.float16))
    vwT = np.ascontiguousarray(np.asarray(v_w, np.float32).T.astype(np.float16))
    maps = []
    for c in range(N_CORES):
        b, half = divmod(c, 2)
        xbc = xf[b] if half == 0 else np.roll(xf[b], -PQ, axis=1)
        xbc = np.ascontiguousarray(xbc.astype(np.float16))
        maps.append({"xb": xbc, "qwT": qwT, "kwT": kwT, "vwT": vwT})
    return maps


def _gather(results):
    z = np.empty((B, DV, HW), np.float32)
    for c in range(N_CORES):
        b, half = divmod(c, 2)
        z[b][:, half * PQ:(half + 1) * PQ] = results[c]["zout"]
    return z.reshape(B, DV, H, W)


def _run(x, q_w, k_w, v_w, trace=False):
    from concourse import bass_utils
    nc = _build()
    res = bass_utils.run_bass_kernel_spmd(
        nc, _in_maps(x, q_w, k_w, v_w), core_ids=list(range(N_CORES)),
        trace=trace)
    return _gather(res.results), res


def kernel(x, q_w, k_w, v_w):
    z, _ = _run(x, q_w, k_w, v_w)
    return z


# revision 36
# speedup vs baseline: 1.2383x; 1.1283x over previous
"""Trainium2 Bass kernel for BasicAttention.

reference math (fp32):
  xf = x.reshape(b, din, hw)               # b=4, din=256, hw=4096
  Q = q_w @ xf   [b, 64, hw]
  K = k_w @ xf   [b, 64, hw]
  V = v_w @ xf   [b, 256, hw]
  S = Q^T K      [b, hw, hw]
  A = softmax(S, axis=-1)
  z = (A @ V^T)^T -> [b, 256, h, w]

Sharding: 8 cores = (batch b in 0..4) x (query half in 0..2). Each core gets
its batch's full xf with columns rotated so its 2048 queries come first
(attention is permutation-invariant over keys, so K/V built from the rotated
xf give identical outputs).

Dtypes: x / weights / Q / K in fp16, S psum fp32, exp -> bf16 (|S| < ~45 so
exp(S) needs bf16's e8 exponent; no max-subtraction pass), V tiles bf16,
Z matmuls bf16 x bf16 -> fp32 psum, output written fp16 (host casts to f32).
End-to-end rel err ~6e-3 vs the 2e-2 gate.

Per-core dataflow:
  - For each 512-query ptile: per 128-key chunk: S^T psum tile [keys=128,
    queries=512] = K_chunk(lhsT) @ Q; exp on ACT straight out of PSUM into
    bf16; two Z matmuls accumulate V^T_chunk^T @ expS into psum [dv=128,512].
  - Projections are emitted just-in-time inside ptile 0's key loop so the
    PE queue never sits behind a long prelude; Q tiles for ptile p are
    emitted at the top of ptile p.
  - Softmax denominators: bf16 pair/quad tree on DVE, fp32 accumulator;
    a ones[128,128] matmul replicates the key-sum across all psum
    partitions; ACT evicts it to SBUF; two DVE divide ops produce the
    normalized fp16 output tiles directly (no reciprocal / broadcast).
  - Each ptile tail is deferred into the next ptile's stream; the last
    ptile's denominator chain is hoisted before its final Z matmuls so only
    the two divides + output DMA sit after the last matmul.
"""

import sys
import os

sys.path.insert(0, "/opt/trn_rl_repo")

import numpy as np

B, DIN, H, W = 4, 256, 64, 64
HW = H * W            # 4096 keys
DK, DV = 64, 256
PQ = HW // 2          # 2048 queries per core
PT = 512              # query tile (psum free dim)
QC = 128              # key chunk (contraction tile)
NPT = PQ // PT        # 4
NQC = HW // QC        # 32
N_CORES = 8

_cache = {}


def _build():
    if "nc" in _cache:
        return _cache["nc"]

    from contextlib import ExitStack
    import concourse.tile as tile
    from concourse import bacc, mybir

    f32 = mybir.dt.float32
    f32r = mybir.dt.float32r
    f16 = mybir.dt.float16
    bf16 = mybir.dt.bfloat16

    nc = bacc.Bacc("TRN2", target_bir_lowering=False, debug=False,
                   num_devices=N_CORES)

    xb = nc.dram_tensor("xb", [DIN, HW], f16, kind="ExternalInput").ap()
    qwT = nc.dram_tensor("qwT", [DIN, DK], f16, kind="ExternalInput").ap()
    kwT = nc.dram_tensor("kwT", [DIN, DK], f16, kind="ExternalInput").ap()
    vwT = nc.dram_tensor("vwT", [DIN, DV], f16, kind="ExternalInput").ap()
    zout = nc.dram_tensor("zout", [DV, PQ], f16, kind="ExternalOutput").ap()

    with tile.TileContext(nc) as tc, ExitStack() as ctx:
        singles = ctx.enter_context(tc.tile_pool(name="singles", bufs=1))
        vt_pool = ctx.enter_context(tc.tile_pool(name="vt_pool", bufs=NQC))
        exps_pool = ctx.enter_context(tc.tile_pool(name="exps_pool", bufs=8))
        sum_pool = ctx.enter_context(tc.tile_pool(name="sum_pool", bufs=2))
        out_pool = ctx.enter_context(tc.tile_pool(name="out_pool", bufs=4))
        ps_s = ctx.enter_context(tc.tile_pool(name="ps_s", bufs=4, space="PSUM"))
        ps_z = ctx.enter_context(tc.tile_pool(name="ps_z", bufs=4, space="PSUM"))

        # ---- weights via SWDGE (parallel with the big x loads below) ----
        w_q0 = singles.tile([128, DK], f16)
        w_q1 = singles.tile([128, DK], f16)
        w_k0 = singles.tile([128, DK], f16)
        w_k1 = singles.tile([128, DK], f16)
        w_v0 = singles.tile([128, DV], f16)
        w_v1 = singles.tile([128, DV], f16)
        # weights on sync/gpsimd queues -- keep the ACT queue empty so the
        # compiler-inserted exp table load runs immediately after the preamble
        nc.sync.dma_start(out=w_k0, in_=kwT[0:128, :])
        nc.sync.dma_start(out=w_k1, in_=kwT[128:256, :])
        nc.gpsimd.dma_start(out=w_q0, in_=qwT[0:128, :])
        nc.gpsimd.dma_start(out=w_q1, in_=qwT[128:256, :])
        nc.gpsimd.dma_start(out=w_v0, in_=vwT[0:128, :])
        nc.gpsimd.dma_start(out=w_v1, in_=vwT[128:256, :])

        # ones weights [128, 128] for the key-sum matmul: the column sums land
        # replicated on every psum partition, so the reciprocal runs on all
        # 128 DVE lanes and feeds the normalize muls directly (no broadcast).
        ones_f = singles.tile([128, 128], f32)
        nc.vector.memset(ones_f, 1.0)
        ones_c = singles.tile([128, 128], f32r)
        nc.scalar.copy(ones_c, ones_f)

        q_sb = singles.tile([DK, PQ], f16)
        k_sb = singles.tile([DK, HW], f16)
        xf0 = singles.tile([128, HW], f16)
        xf1 = singles.tile([128, HW], f16)

        # ---- chunked x load on sync + gpsimd queues (not ACT's) ----
        CHW = 512                        # columns per chunk
        NCH = HW // CHW                  # 8 chunks
        dma_engs = [nc.sync, nc.gpsimd]
        for g in range(NCH):
            sl = slice(g * CHW, (g + 1) * CHW)
            eng = dma_engs[g % 2]
            eng.dma_start(out=xf0[:, sl], in_=xb[0:128, sl])
            eng.dma_start(out=xf1[:, sl], in_=xb[128:256, sl])

        vt = [None] * NQC

        def proj_k(c, pool=None):
            sl = slice(c * CHW, (c + 1) * CHW)
            pool = pool or ps_s
            pk = pool.tile([DK, CHW], f32, name=f"ps_k{c}",
                           tag="ps_s" if pool is ps_s else "pz")
            nc.tensor.matmul(pk, w_k0, xf0[:, sl], start=True, stop=False)
            nc.tensor.matmul(pk, w_k1, xf1[:, sl], start=False, stop=True)
            nc.vector.tensor_copy(k_sb[:, sl], pk)

        def proj_v(c, pool=None):
            pool = pool or ps_s
            for qc in range(c * CHW // QC, (c + 1) * CHW // QC):
                pv = pool.tile([QC, DV], f32, name=f"ps_v{qc}",
                               tag="ps_s" if pool is ps_s else "pz")
                nc.tensor.matmul(pv, xf0[:, qc * QC:(qc + 1) * QC], w_v0,
                                 start=True, stop=False)
                nc.tensor.matmul(pv, xf1[:, qc * QC:(qc + 1) * QC], w_v1,
                                 start=False, stop=True)
                vt_t = vt_pool.tile([QC, DV], bf16, name=f"vt{qc}", tag="vt")
                if qc % 2 == 0:
                    nc.scalar.copy(vt_t, pv)
                else:
                    nc.vector.tensor_copy(vt_t, pv)
                vt[qc] = vt_t

        def proj_q(i):
            sl = slice(i * PT, (i + 1) * PT)
            pq = ps_s.tile([DK, PT], f32, name=f"ps_q{i}", tag="ps_s")
            nc.tensor.matmul(pq, w_q0, xf0[:, sl], start=True, stop=False)
            nc.tensor.matmul(pq, w_q1, xf1[:, sl], start=False, stop=True)
            nc.vector.tensor_copy(q_sb[:, sl], pq)

        proj_k(0)
        proj_q(0)

        # ---- attention main loop ----
        deferred = None
        for pt in range(NPT):
            last_pt = pt == NPT - 1
            qs = q_sb[:, pt * PT:(pt + 1) * PT]
            pz0 = ps_z.tile([128, PT], f32, name=f"pz0_{pt}", tag="pz")
            pz1 = ps_z.tile([128, PT], f32, name=f"pz1_{pt}", tag="pz")
            acc = sum_pool.tile([QC, PT], f32, name=f"acc_{pt}", tag="acc")

            def s_mm(qc, qs=qs, pt=pt):
                ps = ps_s.tile([QC, PT], f32, name=f"ps_{pt}_{qc}", tag="ps_s")
                nc.tensor.matmul(ps, k_sb[:, qc * QC:(qc + 1) * QC], qs,
                                 start=True, stop=True)
                return ps

            pend = [s_mm(i) for i in range(4)]

            def mk_exp(qc, pt=pt):
                e = exps_pool.tile([QC, PT], bf16,
                                   name=f"exps_{pt}_{qc}", tag="exps",
                                   bufs=12)
                nc.scalar.activation(e, pend.pop(0),
                                     func=mybir.ActivationFunctionType.Exp)
                return e

            E = {}
            if pt == 0:
                # upfront projections on PSUM slots borrowed from the ps_z
                # pool (whose accumulators aren't live yet): the ps_s ring
                # stays free for S tiles, so none of this work serializes
                # behind the table-load-gated first exps
                proj_v(0, ps_z)
            E[0], E[1] = mk_exp(0), mk_exp(1)
            if pt == 0:
                proj_k(1, ps_z)
                proj_v(1, ps_z)
            pend.extend([s_mm(4), s_mm(5)])
            E[2], E[3] = mk_exp(2), mk_exp(3)
            if pt == 0:
                proj_k(2, ps_z)
                proj_v(2, ps_z)

            # denominator chain for this ptile: cast, ones-matmul (key sums
            # replicated on all 128 psum partitions), exact reciprocal on
            # 128 DVE lanes straight out of PSUM.
            def den_chain(pt=pt, acc=acc):
                accr = sum_pool.tile([QC, PT], f32r,
                                     name=f"accr{pt}", tag="accr")
                nc.scalar.copy(accr, acc)
                ps_den = ps_s.tile([128, PT], f32,
                                   name=f"ps_den{pt}", tag="ps_s")
                nc.tensor.matmul(ps_den, ones_c, accr, start=True, stop=True)
                den = sum_pool.tile([128, PT], f32, name=f"den{pt}", tag="den")
                # split halves so no single 3.4us DVE op blocks the sum tree
                nc.vector.reciprocal(den[:, 0:PT // 2], ps_den[:, 0:PT // 2])
                nc.vector.reciprocal(den[:, PT // 2:], ps_den[:, PT // 2:])
                return den

            pairs = {}
            den = None

            def sum_tree(g, acc=acc, pairs=pairs, pt=pt, dve=False):
                p_t = exps_pool.tile([QC, PT], bf16,
                                     name=f"pair_{pt}_{g}", tag="pair")
                nc.vector.tensor_add(p_t, E[g], E[g + 1])
                pairs[g] = p_t
                if g % 4 == 2:
                    qd = exps_pool.tile([QC, PT], bf16,
                                        name=f"quad_{pt}_{g}", tag="quad")
                    eng = nc.vector if dve else nc.gpsimd
                    eng.tensor_add(qd, pairs[g - 2], pairs[g])
                    if g == 2:
                        nc.vector.tensor_copy(acc, qd)
                    else:
                        nc.vector.tensor_add(acc, acc, qd)

            if last_pt:
                # run the sum tree as soon as each E pair exists so the
                # denominator chain completes under the final Z matmuls
                sum_tree(0, dve=True)
                sum_tree(2, dve=True)

            for g in range(0, NQC, 2):
                if pt == 0 and g + 6 < NQC and (g + 6) % (CHW // QC) == 0:
                    c = (g + 6) * QC // CHW + 1
                    if c < NCH:
                        proj_k(c)
                        proj_v(c)
                if pt < NPT - 1 and g == 20:
                    # prefetch next ptile's Q projection so its eviction is
                    # done well before the boundary
                    proj_q(pt + 1)
                for h in range(2):
                    if g + 4 + h < NQC:
                        E[g + 4 + h] = mk_exp(g + 4 + h)
                if last_pt and g + 4 < NQC:
                    sum_tree(g + 4, dve=True)
                if last_pt and g == NQC - 6:
                    den = den_chain()
                for v, pz in ((0, pz0), (1, pz1)):
                    hs = (1, 0) if v == 0 else (0, 1)
                    for idx, h in enumerate(hs):
                        nc.tensor.matmul(pz,
                                         vt[g + h][:, v * 128:(v + 1) * 128],
                                         E[g + h],
                                         start=(g == 0 and idx == 0),
                                         stop=(g == NQC - 2 and idx == 1))
                for h in range(2):
                    if g + 6 + h < NQC:
                        pend.append(s_mm(g + 6 + h))
                if not last_pt:
                    sum_tree(g)
                if g == 4 and deferred is not None:
                    deferred()
                    deferred = None

            def make_tail(pt=pt, pz0=pz0, pz1=pz1, den=den, last=last_pt,
                          den_chain=den_chain, acc=acc):
                def tail():
                    d = den if last else den_chain(pt=pt, acc=acc)
                    out0 = out_pool.tile([128, PT], f16,
                                         name=f"out0_{pt}", tag="out")
                    out1 = out_pool.tile([128, PT], f16,
                                         name=f"out1_{pt}", tag="out")
                    hp = PT // 2
                    for lo, hi in ((0, hp), (hp, PT)):
                        nc.vector.tensor_mul(out0[:, lo:hi], pz0[:, lo:hi],
                                             d[:, lo:hi])
                        nc.vector.tensor_mul(out1[:, lo:hi], pz1[:, lo:hi],
                                             d[:, lo:hi])
                    nc.sync.dma_start(out=zout[0:128, pt * PT:(pt + 1) * PT],
                                      in_=out0)
                    nc.sync.dma_start(
                        out=zout[128:256, pt * PT:(pt + 1) * PT], in_=out1)
                return tail

            deferred = make_tail()
        deferred()

    nc.compile()
    _cache["nc"] = nc
    return nc


def _in_maps(x, q_w, k_w, v_w):
    xf = np.asarray(x, np.float32).reshape(B, DIN, HW)
    qwT = np.ascontiguousarray(np.asarray(q_w, np.float32).T.astype(np.float16))
    kwT = np.ascontiguousarray(np.asarray(k_w, np.float32).T.astype(np.float16))
    vwT = np.ascontiguousarray(np.asarray(v_w, np.float32).T.astype(np.float16))
    maps = []
    for c in range(N_CORES):
        b, half = divmod(c, 2)
        xbc = xf[b] if half == 0 else np.roll(xf[b], -PQ, axis=1)
        xbc = np.ascontiguousarray(xbc.astype(np.float16))
        maps.append({"xb": xbc, "qwT": qwT, "kwT": kwT, "vwT": vwT})
    return maps


def _gather(results):
    z = np.empty((B, DV, HW), np.float32)
    for c in range(N_CORES):
        b, half = divmod(c, 2)
        z[b][:, half * PQ:(half + 1) * PQ] = results[c]["zout"]
    return z.reshape(B, DV, H, W)


def _run(x, q_w, k_w, v_w, trace=False):
    from concourse import bass_utils
    nc = _build()
    res = bass_utils.run_bass_kernel_spmd(
        nc, _in_maps(x, q_w, k_w, v_w), core_ids=list(range(N_CORES)),
        trace=trace)
    return _gather(res.results), res


def kernel(x, q_w, k_w, v_w):
    z, _ = _run(x, q_w, k_w, v_w)
    return z
